# revision 21
# baseline (speedup 1.0000x reference)
"""Trainium2 Bass kernel for nn_MixedSparseGatedMLP (LoRA-augmented gated MLP).

Math (reference):
    y1 = x @ Wg + (x @ Ag) @ Bg
    y2 = x @ Wu + (x @ Au) @ Bu
    x3 = relu(y1) * y2
    y3 = x3 @ Wd + (x3 @ Ad) @ Bd

Strategy:
  - Fold the rank-16 LoRA factors into the dense weights on the host
    (exact fp32 algebra): Wg_eff = Wg + Ag@Bg, etc.  The device kernel is
    then a plain gated MLP with three dense matmuls.
  - TP2 x DP4 sharding: I = 11008 = 86*128 chunks split into 2
    tensor-parallel groups of 43 chunks (no padding, unlike an 8-way
    split which needs 88), and tokens split 4 ways (1024 per core, as
    2 blocks of 512).  Core c: group g=c//4, token range d=c%4.
    Each core emits a partial out^T [H, 1024]; the host adds the two
    group partials per token range (gather/unshard step).
  - Down projection runs in out^T orientation: stationary = Wd slice
    [128i x 128h], moving = x3 [128i x 512t], one PSUM tile per
    128-row h-block, 43-deep accumulation.  Wd streams through SBUF
    exactly once per block (it is far too big to keep resident).
  - bf16 operands, fp32 PSUM accumulation, fp32 partial outputs.
  - All DRAM layouts are pre-tiled on the host so every DMA is a linear
    copy into the exact SBUF layout the matmuls need.

Startup (measured on HW):
  - engine sequencers come up at ~6-7.2us, first DMA data lands ~8.6us.
  - warmup matmuls bridge PE activity from ~7.3us (vector-engine memset;
    a scalar-engine memset would serialize behind the 1.3us
    ACT_TABLE_LOAD preamble) so the HAM clock gate reaches 8/8 by the
    time real matmuls start.
  - x and the first 3 chunks' weights stream as fine-grained 256KB
    pieces, need-ordered across 5 DMA issue paths (tensor/vector/sync/
    scalar/gpsimd), so the first real matmul starts at ~10.5us and the
    k-group-interleaved ramp (6 open PSUM groups, ~245GB/s demand)
    stays fed.
"""

import os
import sys

for _p in ("/opt/trn_rl_repo", "/root/.axon_site/_ro/trn_rl_repo"):
    if os.path.isdir(_p) and _p not in sys.path:
        sys.path.append(_p)

import numpy as np
import ml_dtypes

# Problem shapes (hardcoded per contract)
B, S, H, I, R = 2, 2048, 4096, 11008, 16
NTOK = B * S              # 4096 tokens
NCORES = 8
TPG = 2                   # tensor-parallel groups over I
DPG = 4                   # data-parallel groups over tokens
CI = I // 128             # 86 i-chunks total
C = CI // TPG             # 43 i-chunks per core
IS = C * 128              # 5504 intermediate columns per core
K = H // 128              # 32 h-chunks (gate/up contraction)
TB = 512                  # token block
TOKC = NTOK // DPG        # 1024 tokens per core
NB = TOKC // TB           # 2 token blocks per core
NHB = H // 128            # 32 output h-blocks (down phase)
KG = 4                    # k-groups (startup interleave granularity)
KS = K // KG              # 8 k's per group
NXT = K // 2              # 16 x tiles of 2 k's each

BF16 = ml_dtypes.bfloat16

# set by test.py for profiling; harness path leaves these as-is
TRACE = False
LAST_EXEC_TIME_NS = None
LAST_RESULTS = None


def _build_nc():
    import concourse.bacc as bacc
    import concourse.mybir as mybir
    import concourse.tile as tile

    bf16 = mybir.dt.bfloat16
    f32 = mybir.dt.float32

    nc = bacc.Bacc("TRN2", target_bir_lowering=False, debug=False)

    # DRAM parameters (host pre-tiled layouts; see kernel() for the math).
    # x is piece-tiled [block, 2-k piece, partition, cols] and wq0 holds
    # the startup chunks' weights piece-tiled per (chunk, branch,
    # k-group) so every startup DMA is a CONTIGUOUS 256KB read (strided
    # 2KB-row reads measured only ~60-90GB/s vs ~146 contiguous).
    x = nc.declare_dram_parameter("x", [NB, NXT, 128, 2 * TB], bf16, isOutput=False)
    wg = nc.declare_dram_parameter("wg", [C, 128, K * 128], bf16, isOutput=False)
    wu = nc.declare_dram_parameter("wu", [C, 128, K * 128], bf16, isOutput=False)
    wq0 = nc.declare_dram_parameter("wq0", [3, 2, KG, 128, KS * 128], bf16,
                                    isOutput=False)
    wd = nc.declare_dram_parameter("wd", [NHB, 128, C * 128], bf16, isOutput=False)
    out = nc.declare_dram_parameter("out", [H, TOKC], f32, isOutput=True)

    SC = 3   # startup chunks processed k-interleaved

    with tile.TileContext(nc) as tc:
        with tc.tile_pool(name="xp", bufs=1) as xp, \
             tc.tile_pool(name="wp", bufs=4) as wp, \
             tc.tile_pool(name="wqp", bufs=1) as wqp, \
             tc.tile_pool(name="wdp", bufs=3) as wdp, \
             tc.tile_pool(name="x3p", bufs=1) as x3p, \
             tc.tile_pool(name="rp", bufs=2) as rp, \
             tc.tile_pool(name="op", bufs=4) as op, \
             tc.tile_pool(name="pgp", bufs=3, space="PSUM") as pgp, \
             tc.tile_pool(name="pup", bufs=3, space="PSUM") as pup, \
             tc.tile_pool(name="pdp", bufs=2, space="PSUM") as pdp:

            # x lives in SBUF as 16 tiles of 2 k-chunks each; fine
            # granularity exists so startup matmuls can begin after the
            # first 256KB lands instead of after a whole 1MB tile.
            def x_piece(b, t, eng):
                tl = xp.tile([128, 2 * TB], bf16, tag=f"x{t}")
                eng.dma_start(tl, x[b][t])
                return tl

            def xk(xcur, k):
                t, j = divmod(k, 2)
                return xcur[t][:, j * TB:(j + 1) * TB]

            # DMA issue paths: sync + scalar (HWDGE) and gpsimd (SWDGE).
            # gpsimd is the fastest path under contention (~150GB/s vs
            # ~75-95 for each HWDGE queue; ~330GB/s aggregate cap).
            xcur = [None] * NXT
            xcur[0] = x_piece(0, 0, nc.gpsimd)

            # PE warmup: dependency-free matmuls lift the HAM clock gate
            # to 8/8 before real work.  memset on the VECTOR engine: it
            # is ready ~3us before scalar (ACT_TABLE_LOAD) and gpsimd.
            warm_in = rp.tile([128, TB], bf16, tag="warm")
            nc.vector.memset(warm_in, 0.0)
            warm_ps = pdp.tile([128, TB], f32, tag="pd")
            for _ in range(9):
                nc.tensor.matmul(warm_ps, warm_in[:, 0:128], warm_in,
                                 start=True, stop=True)

            def warm_mm():
                # filler matmul: keeps the PE (and the HAM activity
                # window) busy across a just-in-time DMA arrival jitter
                # during the ramp instead of idling into a re-throttle.
                nc.tensor.matmul(warm_ps, warm_in[:, 0:128], warm_in,
                                 start=True, stop=True)

            # startup weight pieces: [128, KS*128] = 256KB per (chunk,
            # branch, k-group).  kg 2/3 reuse kg 0/1's tags -- the
            # write-after-read dependency self-times their DMAs.
            wq = {}

            def wq_piece(c, w, kg, eng):
                wi = 0 if w == 'g' else 1
                tl = wqp.tile([128, KS * 128], bf16, tag=f"wq{c}{w}{kg % 2}")
                eng.dma_start(tl, wq0[c][wi][kg])
                wq[(c, w, kg)] = tl

            # need-ordered startup streams: pieces are emitted in global
            # consumption order (keeps tag-ring reuse aligned) and
            # assigned across the three DMA issue paths weighted by
            # their measured contended bandwidth (gpsimd every other
            # piece).  sync data flows from ~8.6us, gpsimd ~8.7us,
            # scalar ~9.9us (behind ACT_TABLE_LOAD).
            engs = [nc.sync, nc.gpsimd, nc.scalar, nc.gpsimd]
            ei = 0   # xt0 already went out on gpsimd

            def nxt_eng():
                nonlocal ei
                e = engs[ei % 4]
                ei += 1
                return e

            need_order = []
            for kg in range(KG):
                need_order.append(("x", kg * 4))
                for c in range(SC):
                    need_order.append(("w", c, 'g', kg))
                    need_order.append(("w", c, 'u', kg))
                for p in (1, 2, 3):
                    need_order.append(("x", kg * 4 + p))
            # gpsimd (every other slot) finishes its ~5MB share ~4us
            # after sync/scalar drain theirs; shift two late pieces to
            # the by-then-idle queues so kg3 isn't gated on gpsimd.
            override = {36: nc.scalar, 38: nc.sync}
            for idx, item in enumerate(need_order):
                if item[0] == "x":
                    t = item[1]
                    if xcur[t] is None:
                        xcur[t] = x_piece(0, t, override.get(idx) or nxt_eng())
                else:
                    _, c, w, kg = item
                    wq_piece(c, w, kg, override.get(idx) or nxt_eng())

            # x3^T for this block: [128 i-in-chunk, (c, t)] bf16
            def new_x3():
                return x3p.tile([128, C * TB], bf16, tag="x3", name="x3")

            x3 = new_x3()

            # ---- interleaved startup: chunks 0..SC-1 ----
            # 6 PSUM groups (3 chunks x gate/up) accumulate in kk-pair
            # waves so weight+x demand (~245GB/s) tracks DMA delivery.
            psg = [pgp.tile([128, TB], f32, tag="pg", name=f"psg{c}")
                   for c in range(SC)]
            psu = [pup.tile([128, TB], f32, tag="pu", name=f"psu{c}")
                   for c in range(SC)]
            for kg in range(KG):
                for p in range(KS // 2):
                    for c in range(SC):
                        for w in ('g', 'u'):
                            ps = psg[c] if w == 'g' else psu[c]
                            t = wq[(c, w, kg)]
                            for j in range(2):
                                kk = 2 * p + j
                                nc.tensor.matmul(
                                    ps,
                                    t[:, kk * 128:(kk + 1) * 128],
                                    xk(xcur, kg * KS + kk),
                                    start=(kg == 0 and kk == 0),
                                    stop=(kg == KG - 1 and kk == KS - 1),
                                )
                            if kg == 0 and p == 0:
                                warm_mm()
                        if kg == KG - 1 and p == KS // 2 - 1:
                            r = rp.tile([128, TB], bf16, tag="r")
                            nc.scalar.activation(
                                r, psg[c], mybir.ActivationFunctionType.Relu)
                            nc.vector.tensor_mul(
                                x3[:, c * TB:(c + 1) * TB], r, psu[c])

            def w_tile(src, m, eng):
                # monolithic 1MB weight DMAs: large transfers sustain
                # high per-queue bandwidth; wg rides HWDGE (sync), wu
                # rides SWDGE (gpsimd) so each queue runs at ~73GB/s
                # with slack instead of one queue at its 146GB/s limit.
                t = wp.tile([128, K * 128], bf16, tag="w")
                eng.dma_start(t, src[m])
                return t

            def wd_tile(n, eng):
                t = wdp.tile([128, C * 128], bf16, tag="wdn")
                eng.dma_start(t, wd[n])
                return t

            wpref = {}
            wdpref = {}
            xnext = [None] * NXT

            for b in range(NB):
                if b > 0:
                    # prefetched during the previous down phase
                    xcur = list(xnext)
                    x3 = new_x3()

                # ---- gate / up projections + gating, per i-chunk m ----
                for m in range(SC if b == 0 else 0, C):
                    if (b, m) in wpref:
                        wgt, wut = wpref.pop((b, m))
                    else:
                        wgt = w_tile(wg, m, nc.sync)
                        wut = w_tile(wu, m, nc.gpsimd)

                    # wd prefetch for the down phase: on gpsimd (the
                    # fast path, lightly loaded during gate phases),
                    # spread out so it doesn't contend with the
                    # startup/weight streams.
                    base = SC if b == 0 else 0
                    if m in (base, base + 4, base + 8):
                        n = (m - base) // 4
                        wdpref[(b, n)] = wd_tile(n, nc.gpsimd)

                    g = pgp.tile([128, TB], f32, tag="pg")
                    u = pup.tile([128, TB], f32, tag="pu")
                    for k in range(K):
                        nc.tensor.matmul(
                            g, wgt[:, k * 128:(k + 1) * 128], xk(xcur, k),
                            start=(k == 0), stop=(k == K - 1),
                        )
                    for k in range(K):
                        nc.tensor.matmul(
                            u, wut[:, k * 128:(k + 1) * 128], xk(xcur, k),
                            start=(k == 0), stop=(k == K - 1),
                        )
                    # x3 = relu(g) * u ; DVE may read only one PSUM input,
                    # so relu lands in SBUF via ACT first.
                    r = rp.tile([128, TB], bf16, tag="r")
                    nc.scalar.activation(r, g, mybir.ActivationFunctionType.Relu)
                    nc.vector.tensor_mul(x3[:, m * TB:(m + 1) * TB], r, u)

                # ---- down projection, out^T orientation ----
                # psum [128 h, TB tok] accumulates over all 43 i-chunks;
                # wd alternates between the sync and scalar queues
                # (~74GB/s each, JIT via wdp backpressure), stores ride
                # SWDGE.
                for n in range(NHB):
                    wdt = wdpref.pop((b, n), None)
                    if wdt is None:
                        wdt = wd_tile(n, nc.sync if n % 2 == 0 else nc.scalar)
                    d = pdp.tile([128, TB], f32, tag="pd")
                    for c in range(C):
                        nc.tensor.matmul(
                            d,
                            wdt[:, c * 128:(c + 1) * 128],
                            x3[:, c * TB:(c + 1) * TB],
                            start=(c == 0), stop=(c == C - 1),
                        )
                    if b == NB - 1 and n == NHB - 1:
                        # tail: split the final copy+store across two
                        # engines and two HWDGE queues (NOT gpsimd --
                        # a store there puts its slow queue-drain on
                        # the teardown critical path) so the last bytes
                        # land ~2us after the last matmul.
                        HT = TB // 2
                        o0 = op.tile([128, HT], f32, tag="o0")
                        o1 = op.tile([128, HT], f32, tag="o1")
                        nc.scalar.copy(o0, d[:, 0:HT])
                        nc.vector.tensor_copy(o1, d[:, HT:TB])
                        r0 = out[n * 128:(n + 1) * 128, b * TB:b * TB + HT]
                        r1 = out[n * 128:(n + 1) * 128, b * TB + HT:(b + 1) * TB]
                        nc.sync.dma_start(r0, o0)
                        nc.scalar.dma_start(r1, o1)
                    else:
                        o = op.tile([128, TB], f32, tag="o")
                        nc.scalar.copy(o, d)
                        nc.gpsimd.dma_start(
                            out[n * 128:(n + 1) * 128, b * TB:(b + 1) * TB], o
                        )

                    if b < NB - 1:
                        # prefetch the next block's x and first gate/up
                        # weights on SWDGE while wd streams on sync+scalar
                        if 2 <= n < 2 + NXT:
                            xnext[n - 2] = x_piece(b + 1, n - 2, nc.gpsimd)
                        elif n == 26:
                            wgt1 = w_tile(wg, 0, nc.gpsimd)
                        elif n == 28:
                            wut1 = w_tile(wu, 0, nc.gpsimd)
                            wpref[(b + 1, 0)] = (wgt1, wut1)

    nc.compile()
    return nc


def _prep_inputs(x1, w_gate, w_gate_lora_a, w_gate_lora_b,
                 w_up, w_up_lora_a, w_up_lora_b,
                 w_down, w_down_lora_a, w_down_lora_b):
    """Fold LoRA, shard TP2xDP4, and pre-tile DRAM layouts."""
    f32 = np.float32
    x1 = np.asarray(x1, f32)
    wg_eff = np.asarray(w_gate, f32) + np.asarray(w_gate_lora_a, f32) @ np.asarray(w_gate_lora_b, f32)
    wu_eff = np.asarray(w_up, f32) + np.asarray(w_up_lora_a, f32) @ np.asarray(w_up_lora_b, f32)
    wd_eff = np.asarray(w_down, f32) + np.asarray(w_down_lora_a, f32) @ np.asarray(w_down_lora_b, f32)

    x2d = x1.reshape(NTOK, H)

    # per-group weight tilings
    gmaps = []
    for g in range(TPG):
        sl = slice(g * IS, (g + 1) * IS)
        # wg tile layout: [m, p, k*128+j] = wg_eff[k*128+p, g*IS + m*128 + j]
        wgc = np.ascontiguousarray(
            wg_eff[:, sl].reshape(K, 128, C, 128).transpose(2, 1, 0, 3)
        ).astype(BF16).reshape(C, 128, K * 128)
        wuc = np.ascontiguousarray(
            wu_eff[:, sl].reshape(K, 128, C, 128).transpose(2, 1, 0, 3)
        ).astype(BF16).reshape(C, 128, K * 128)
        # wd tile layout: [n, p, c*128+h] = wd_eff[g*IS + c*128 + p, n*128+h]
        wdc = np.ascontiguousarray(
            wd_eff[sl, :].reshape(C, 128, NHB, 128).transpose(2, 1, 0, 3)
        ).astype(BF16).reshape(NHB, 128, C * 128)
        gmaps.append((wgc, wuc, wdc))

    # per-DP-slice x tilings, piece-tiled so every x DMA is contiguous:
    # x_t[b, t2, p, j*TB+t] = x2d[d*TOKC + b*TB + t, (2*t2+j)*128+p]
    xmaps = []
    for d in range(DPG):
        xs = x2d[d * TOKC:(d + 1) * TOKC]
        xt = np.ascontiguousarray(
            xs.reshape(NB, TB, NXT, 2, 128).transpose(0, 2, 4, 3, 1)
        ).astype(BF16).reshape(NB, NXT, 128, 2 * TB)
        xmaps.append(xt)

    in_maps = []
    for ci in range(NCORES):
        g, d = divmod(ci, DPG)
        wgc, wuc, wdc = gmaps[g]
        # startup weight pieces, contiguous per (chunk, branch, k-group)
        wq0 = np.empty((3, 2, KG, 128, KS * 128), dtype=BF16)
        for c in range(3):
            for wi, wsrc in enumerate((wgc, wuc)):
                for kg in range(KG):
                    wq0[c, wi, kg] = wsrc[c][:, kg * KS * 128:(kg + 1) * KS * 128]
        in_maps.append({"x": xmaps[d], "wg": wgc, "wu": wuc, "wq0": wq0,
                        "wd": wdc})
    return in_maps


def _emulate(in_maps):
    """Numpy emulation of the device math (bf16 operands, fp32 accum),
    reconstructing operands from the tiled layouts to validate them."""
    f32 = np.float32
    acc = np.zeros((NTOK, H), f32)
    for ci, m in enumerate(in_maps):
        g, d = divmod(ci, DPG)
        xt = m["x"].reshape(NB, NXT, 128, 2, TB)
        xs = xt.transpose(0, 4, 1, 3, 2).reshape(TOKC, H).astype(f32)
        wgc = m["wg"].reshape(C, 128, K, 128)
        wg2 = wgc.transpose(2, 1, 0, 3).reshape(H, IS).astype(f32)
        wuc = m["wu"].reshape(C, 128, K, 128)
        wu2 = wuc.transpose(2, 1, 0, 3).reshape(H, IS).astype(f32)
        wdc = m["wd"].reshape(NHB, 128, C, 128)
        wd2 = wdc.transpose(2, 1, 0, 3).reshape(IS, H).astype(f32)
        y1 = xs @ wg2
        y2 = xs @ wu2
        r = np.maximum(y1, 0).astype(BF16).astype(f32)
        x3 = (r * y2).astype(BF16).astype(f32)
        acc[d * TOKC:(d + 1) * TOKC] += x3 @ wd2
    return acc.reshape(B, S, H)


def kernel(**inputs):
    global LAST_EXEC_TIME_NS, LAST_RESULTS
    in_maps = _prep_inputs(**inputs)

    if os.environ.get("KERNEL_EMULATE"):
        return _emulate(in_maps)

    from concourse.bass_utils import run_bass_kernel_spmd

    nc = _build_nc()
    res = run_bass_kernel_spmd(nc, in_maps, list(range(NCORES)), trace=TRACE)
    LAST_EXEC_TIME_NS = res.exec_time_ns
    LAST_RESULTS = res

    acc = np.zeros((NTOK, H), np.float32)
    for ci, r in enumerate(res.results):
        g, d = divmod(ci, DPG)
        acc[d * TOKC:(d + 1) * TOKC] += r["out"].T
    return acc.reshape(B, S, H)


# revision 23
# speedup vs baseline: 1.0009x; 1.0009x over previous
"""Trainium2 Bass kernel for nn_MixedSparseGatedMLP (LoRA-augmented gated MLP).

Math (reference):
    y1 = x @ Wg + (x @ Ag) @ Bg
    y2 = x @ Wu + (x @ Au) @ Bu
    x3 = relu(y1) * y2
    y3 = x3 @ Wd + (x3 @ Ad) @ Bd

Strategy:
  - Fold the rank-16 LoRA factors into the dense weights on the host
    (exact fp32 algebra): Wg_eff = Wg + Ag@Bg, etc.  The device kernel is
    then a plain gated MLP with three dense matmuls.
  - TP2 x DP4 sharding: I = 11008 = 86*128 chunks split into 2
    tensor-parallel groups of 43 chunks (no padding, unlike an 8-way
    split which needs 88), and tokens split 4 ways (1024 per core, as
    2 blocks of 512).  Core c: group g=c//4, token range d=c%4.
    Each core emits a partial out^T [H, 1024]; the host adds the two
    group partials per token range (gather/unshard step).
  - Down projection runs in out^T orientation: stationary = Wd slice
    [128i x 128h], moving = x3 [128i x 512t], one PSUM tile per
    128-row h-block, 43-deep accumulation.  Wd streams through SBUF
    exactly once per block (it is far too big to keep resident).
  - bf16 operands, fp32 PSUM accumulation, fp32 partial outputs.
  - All DRAM layouts are pre-tiled on the host so every DMA is a linear
    copy into the exact SBUF layout the matmuls need.

Startup (measured on HW):
  - engine sequencers come up at ~6-7.2us, first DMA data lands ~8.6us.
  - warmup matmuls bridge PE activity from ~7.3us (vector-engine memset;
    a scalar-engine memset would serialize behind the 1.3us
    ACT_TABLE_LOAD preamble) so the HAM clock gate reaches 8/8 by the
    time real matmuls start.
  - x and the first 3 chunks' weights stream as fine-grained 256KB
    pieces, need-ordered across 5 DMA issue paths (tensor/vector/sync/
    scalar/gpsimd), so the first real matmul starts at ~10.5us and the
    k-group-interleaved ramp (6 open PSUM groups, ~245GB/s demand)
    stays fed.
"""

import os
import sys

for _p in ("/opt/trn_rl_repo", "/root/.axon_site/_ro/trn_rl_repo"):
    if os.path.isdir(_p) and _p not in sys.path:
        sys.path.append(_p)

import numpy as np
import ml_dtypes

# Problem shapes (hardcoded per contract)
B, S, H, I, R = 2, 2048, 4096, 11008, 16
NTOK = B * S              # 4096 tokens
NCORES = 8
TPG = 2                   # tensor-parallel groups over I
DPG = 4                   # data-parallel groups over tokens
CI = I // 128             # 86 i-chunks total
C = CI // TPG             # 43 i-chunks per core
IS = C * 128              # 5504 intermediate columns per core
K = H // 128              # 32 h-chunks (gate/up contraction)
TB = 512                  # token block
TOKC = NTOK // DPG        # 1024 tokens per core
NB = TOKC // TB           # 2 token blocks per core
NHB = H // 128            # 32 output h-blocks (down phase)
KG = 4                    # k-groups (startup interleave granularity)
KS = K // KG              # 8 k's per group
NXT = K // 2              # 16 x tiles of 2 k's each

BF16 = ml_dtypes.bfloat16

# set by test.py for profiling; harness path leaves these as-is
TRACE = False
LAST_EXEC_TIME_NS = None
LAST_RESULTS = None


def _build_nc():
    import concourse.bacc as bacc
    import concourse.mybir as mybir
    import concourse.tile as tile

    bf16 = mybir.dt.bfloat16
    f32 = mybir.dt.float32

    nc = bacc.Bacc("TRN2", target_bir_lowering=False, debug=False)

    # DRAM parameters (host pre-tiled layouts; see kernel() for the math).
    # x is piece-tiled [block, 2-k piece, partition, cols] and wq0 holds
    # the startup chunks' weights piece-tiled per (chunk, branch,
    # k-group) so every startup DMA is a CONTIGUOUS 256KB read (strided
    # 2KB-row reads measured only ~60-90GB/s vs ~146 contiguous).
    x = nc.declare_dram_parameter("x", [NB, NXT, 128, 2 * TB], bf16, isOutput=False)
    wg = nc.declare_dram_parameter("wg", [C, 128, K * 128], bf16, isOutput=False)
    wu = nc.declare_dram_parameter("wu", [C, 128, K * 128], bf16, isOutput=False)
    wq0 = nc.declare_dram_parameter("wq0", [3, 2, KG, 128, KS * 128], bf16,
                                    isOutput=False)
    wd = nc.declare_dram_parameter("wd", [NHB, 128, C * 128], bf16, isOutput=False)
    out = nc.declare_dram_parameter("out", [H, TOKC], f32, isOutput=True)

    SC = 3   # startup chunks processed k-interleaved

    with tile.TileContext(nc) as tc:
        with tc.tile_pool(name="xp", bufs=1) as xp, \
             tc.tile_pool(name="wp", bufs=4) as wp, \
             tc.tile_pool(name="wqp", bufs=1) as wqp, \
             tc.tile_pool(name="wdp", bufs=3) as wdp, \
             tc.tile_pool(name="x3p", bufs=1) as x3p, \
             tc.tile_pool(name="rp", bufs=2) as rp, \
             tc.tile_pool(name="op", bufs=4) as op, \
             tc.tile_pool(name="pgp", bufs=3, space="PSUM") as pgp, \
             tc.tile_pool(name="pup", bufs=3, space="PSUM") as pup, \
             tc.tile_pool(name="pdp", bufs=2, space="PSUM") as pdp:

            # x lives in SBUF as 16 tiles of 2 k-chunks each; fine
            # granularity exists so startup matmuls can begin after the
            # first 256KB lands instead of after a whole 1MB tile.
            def x_piece(b, t, eng):
                tl = xp.tile([128, 2 * TB], bf16, tag=f"x{t}")
                eng.dma_start(tl, x[b][t])
                return tl

            def xk(xcur, k):
                t, j = divmod(k, 2)
                return xcur[t][:, j * TB:(j + 1) * TB]

            # DMA issue paths: sync + scalar (HWDGE) and gpsimd (SWDGE).
            # gpsimd is the fastest path under contention (~150GB/s vs
            # ~75-95 for each HWDGE queue; ~330GB/s aggregate cap).
            xcur = [None] * NXT
            xcur[0] = x_piece(0, 0, nc.gpsimd)

            # PE warmup: dependency-free matmuls lift the HAM clock gate
            # to 8/8 before real work.  memset on the VECTOR engine: it
            # is ready ~3us before scalar (ACT_TABLE_LOAD) and gpsimd.
            warm_in = rp.tile([128, TB], bf16, tag="warm")
            nc.vector.memset(warm_in, 0.0)
            warm_ps = pdp.tile([128, TB], f32, tag="pd")
            for _ in range(9):
                nc.tensor.matmul(warm_ps, warm_in[:, 0:128], warm_in,
                                 start=True, stop=True)

            def warm_mm():
                # filler matmul: keeps the PE (and the HAM activity
                # window) busy across a just-in-time DMA arrival jitter
                # during the ramp instead of idling into a re-throttle.
                nc.tensor.matmul(warm_ps, warm_in[:, 0:128], warm_in,
                                 start=True, stop=True)

            # startup weight pieces: [128, KS*128] = 256KB per (chunk,
            # branch, k-group).  kg 2/3 reuse kg 0/1's tags -- the
            # write-after-read dependency self-times their DMAs.
            wq = {}

            def wq_piece(c, w, kg, eng):
                wi = 0 if w == 'g' else 1
                tl = wqp.tile([128, KS * 128], bf16, tag=f"wq{c}{w}{kg % 2}")
                eng.dma_start(tl, wq0[c][wi][kg])
                wq[(c, w, kg)] = tl

            # need-ordered startup streams: pieces are emitted in global
            # consumption order (keeps tag-ring reuse aligned) and
            # assigned across the three DMA issue paths weighted by
            # their measured contended bandwidth (gpsimd every other
            # piece).  sync data flows from ~8.6us, gpsimd ~8.7us,
            # scalar ~9.9us (behind ACT_TABLE_LOAD).
            engs = [nc.sync, nc.gpsimd, nc.scalar, nc.gpsimd]
            ei = 0   # xt0 already went out on gpsimd

            def nxt_eng():
                nonlocal ei
                e = engs[ei % 4]
                ei += 1
                return e

            need_order = []
            for kg in range(KG):
                need_order.append(("x", kg * 4))
                for c in range(SC):
                    need_order.append(("w", c, 'g', kg))
                    need_order.append(("w", c, 'u', kg))
                for p in (1, 2, 3):
                    need_order.append(("x", kg * 4 + p))
            # gpsimd (every other slot) would otherwise finish its ~5MB
            # share ~4us after sync/scalar drain theirs; shift late kg3
            # pieces around so no single queue gates the ramp tail.
            override = {34: nc.scalar, 36: nc.scalar, 38: nc.sync,
                        39: nc.gpsimd}
            for idx, item in enumerate(need_order):
                if item[0] == "x":
                    t = item[1]
                    if xcur[t] is None:
                        eng = nxt_eng()
                        xcur[t] = x_piece(0, t, override.get(idx, eng))
                else:
                    _, c, w, kg = item
                    eng = nxt_eng()
                    wq_piece(c, w, kg, override.get(idx, eng))

            # x3^T for this block: [128 i-in-chunk, (c, t)] bf16
            def new_x3():
                return x3p.tile([128, C * TB], bf16, tag="x3", name="x3")

            x3 = new_x3()

            # ---- interleaved startup: chunks 0..SC-1 ----
            # 6 PSUM groups (3 chunks x gate/up) accumulate in kk-pair
            # waves so weight+x demand (~245GB/s) tracks DMA delivery.
            psg = [pgp.tile([128, TB], f32, tag="pg", name=f"psg{c}")
                   for c in range(SC)]
            psu = [pup.tile([128, TB], f32, tag="pu", name=f"psu{c}")
                   for c in range(SC)]
            for kg in range(KG):
                for p in range(KS // 2):
                    for c in range(SC):
                        for w in ('g', 'u'):
                            ps = psg[c] if w == 'g' else psu[c]
                            t = wq[(c, w, kg)]
                            for j in range(2):
                                kk = 2 * p + j
                                nc.tensor.matmul(
                                    ps,
                                    t[:, kk * 128:(kk + 1) * 128],
                                    xk(xcur, kg * KS + kk),
                                    start=(kg == 0 and kk == 0),
                                    stop=(kg == KG - 1 and kk == KS - 1),
                                )
                            if kg == 0 and p == 0:
                                warm_mm()
                        if kg == KG - 1 and p == KS // 2 - 1:
                            r = rp.tile([128, TB], bf16, tag="r")
                            nc.scalar.activation(
                                r, psg[c], mybir.ActivationFunctionType.Relu)
                            nc.vector.tensor_mul(
                                x3[:, c * TB:(c + 1) * TB], r, psu[c])

            def w_tile(src, m, eng):
                # monolithic 1MB weight DMAs: large transfers sustain
                # high per-queue bandwidth; wg rides HWDGE (sync), wu
                # rides SWDGE (gpsimd) so each queue runs at ~73GB/s
                # with slack instead of one queue at its 146GB/s limit.
                t = wp.tile([128, K * 128], bf16, tag="w")
                eng.dma_start(t, src[m])
                return t

            def wd_tile(n, eng):
                t = wdp.tile([128, C * 128], bf16, tag="wdn")
                eng.dma_start(t, wd[n])
                return t

            wpref = {}
            wdpref = {}
            xnext = [None] * NXT

            for b in range(NB):
                if b > 0:
                    # prefetched during the previous down phase
                    xcur = list(xnext)
                    x3 = new_x3()

                # ---- gate / up projections + gating, per i-chunk m ----
                for m in range(SC if b == 0 else 0, C):
                    if (b, m) in wpref:
                        wgt, wut = wpref.pop((b, m))
                    else:
                        # chunks 3-4 of block 0 land while sync is still
                        # draining ramp pieces; the scalar queue is idle
                        # by then and delivers them without a stall.
                        weng = nc.scalar if (b == 0 and m in (3, 4)) else nc.sync
                        wgt = w_tile(wg, m, weng)
                        wut = w_tile(wu, m, nc.gpsimd)

                    # wd prefetch for the down phase: on gpsimd (the
                    # fast path, lightly loaded during gate phases),
                    # spread out so it doesn't contend with the
                    # startup/weight streams.
                    base = SC if b == 0 else 0
                    if m in (base, base + 4, base + 8):
                        n = (m - base) // 4
                        wdpref[(b, n)] = wd_tile(n, nc.gpsimd)

                    g = pgp.tile([128, TB], f32, tag="pg")
                    u = pup.tile([128, TB], f32, tag="pu")
                    for k in range(K):
                        nc.tensor.matmul(
                            g, wgt[:, k * 128:(k + 1) * 128], xk(xcur, k),
                            start=(k == 0), stop=(k == K - 1),
                        )
                    for k in range(K):
                        nc.tensor.matmul(
                            u, wut[:, k * 128:(k + 1) * 128], xk(xcur, k),
                            start=(k == 0), stop=(k == K - 1),
                        )
                    # x3 = relu(g) * u ; DVE may read only one PSUM input,
                    # so relu lands in SBUF via ACT first.
                    r = rp.tile([128, TB], bf16, tag="r")
                    nc.scalar.activation(r, g, mybir.ActivationFunctionType.Relu)
                    nc.vector.tensor_mul(x3[:, m * TB:(m + 1) * TB], r, u)

                # ---- down projection, out^T orientation ----
                # psum [128 h, TB tok] accumulates over all 43 i-chunks;
                # wd alternates between the sync and scalar queues
                # (~74GB/s each, JIT via wdp backpressure), stores ride
                # SWDGE.
                for n in range(NHB):
                    wdt = wdpref.pop((b, n), None)
                    if wdt is None:
                        wdt = wd_tile(n, nc.sync if n % 2 == 0 else nc.scalar)
                    d = pdp.tile([128, TB], f32, tag="pd")
                    for c in range(C):
                        nc.tensor.matmul(
                            d,
                            wdt[:, c * 128:(c + 1) * 128],
                            x3[:, c * TB:(c + 1) * TB],
                            start=(c == 0), stop=(c == C - 1),
                        )
                    if b == NB - 1 and n == NHB - 1:
                        # tail: split the final copy+store across two
                        # engines and two HWDGE queues (NOT gpsimd --
                        # a store there puts its slow queue-drain on
                        # the teardown critical path) so the last bytes
                        # land ~2us after the last matmul.
                        HT = TB // 2
                        o0 = op.tile([128, HT], f32, tag="o0")
                        o1 = op.tile([128, HT], f32, tag="o1")
                        nc.scalar.copy(o0, d[:, 0:HT])
                        nc.vector.tensor_copy(o1, d[:, HT:TB])
                        r0 = out[n * 128:(n + 1) * 128, b * TB:b * TB + HT]
                        r1 = out[n * 128:(n + 1) * 128, b * TB + HT:(b + 1) * TB]
                        nc.sync.dma_start(r0, o0)
                        nc.scalar.dma_start(r1, o1)
                    else:
                        o = op.tile([128, TB], f32, tag="o")
                        nc.scalar.copy(o, d)
                        nc.gpsimd.dma_start(
                            out[n * 128:(n + 1) * 128, b * TB:(b + 1) * TB], o
                        )

                    if b < NB - 1:
                        # prefetch the next block's x and first gate/up
                        # weights on SWDGE while wd streams on sync+scalar
                        if 2 <= n < 2 + NXT:
                            xnext[n - 2] = x_piece(b + 1, n - 2, nc.gpsimd)
                        elif n == 26:
                            wgt1 = w_tile(wg, 0, nc.gpsimd)
                        elif n == 28:
                            wut1 = w_tile(wu, 0, nc.gpsimd)
                            wpref[(b + 1, 0)] = (wgt1, wut1)

    nc.compile()
    return nc


def _prep_inputs(x1, w_gate, w_gate_lora_a, w_gate_lora_b,
                 w_up, w_up_lora_a, w_up_lora_b,
                 w_down, w_down_lora_a, w_down_lora_b):
    """Fold LoRA, shard TP2xDP4, and pre-tile DRAM layouts."""
    f32 = np.float32
    x1 = np.asarray(x1, f32)
    wg_eff = np.asarray(w_gate, f32) + np.asarray(w_gate_lora_a, f32) @ np.asarray(w_gate_lora_b, f32)
    wu_eff = np.asarray(w_up, f32) + np.asarray(w_up_lora_a, f32) @ np.asarray(w_up_lora_b, f32)
    wd_eff = np.asarray(w_down, f32) + np.asarray(w_down_lora_a, f32) @ np.asarray(w_down_lora_b, f32)

    x2d = x1.reshape(NTOK, H)

    # per-group weight tilings
    gmaps = []
    for g in range(TPG):
        sl = slice(g * IS, (g + 1) * IS)
        # wg tile layout: [m, p, k*128+j] = wg_eff[k*128+p, g*IS + m*128 + j]
        wgc = np.ascontiguousarray(
            wg_eff[:, sl].reshape(K, 128, C, 128).transpose(2, 1, 0, 3)
        ).astype(BF16).reshape(C, 128, K * 128)
        wuc = np.ascontiguousarray(
            wu_eff[:, sl].reshape(K, 128, C, 128).transpose(2, 1, 0, 3)
        ).astype(BF16).reshape(C, 128, K * 128)
        # wd tile layout: [n, p, c*128+h] = wd_eff[g*IS + c*128 + p, n*128+h]
        wdc = np.ascontiguousarray(
            wd_eff[sl, :].reshape(C, 128, NHB, 128).transpose(2, 1, 0, 3)
        ).astype(BF16).reshape(NHB, 128, C * 128)
        gmaps.append((wgc, wuc, wdc))

    # per-DP-slice x tilings, piece-tiled so every x DMA is contiguous:
    # x_t[b, t2, p, j*TB+t] = x2d[d*TOKC + b*TB + t, (2*t2+j)*128+p]
    xmaps = []
    for d in range(DPG):
        xs = x2d[d * TOKC:(d + 1) * TOKC]
        xt = np.ascontiguousarray(
            xs.reshape(NB, TB, NXT, 2, 128).transpose(0, 2, 4, 3, 1)
        ).astype(BF16).reshape(NB, NXT, 128, 2 * TB)
        xmaps.append(xt)

    in_maps = []
    for ci in range(NCORES):
        g, d = divmod(ci, DPG)
        wgc, wuc, wdc = gmaps[g]
        # startup weight pieces, contiguous per (chunk, branch, k-group)
        wq0 = np.empty((3, 2, KG, 128, KS * 128), dtype=BF16)
        for c in range(3):
            for wi, wsrc in enumerate((wgc, wuc)):
                for kg in range(KG):
                    wq0[c, wi, kg] = wsrc[c][:, kg * KS * 128:(kg + 1) * KS * 128]
        in_maps.append({"x": xmaps[d], "wg": wgc, "wu": wuc, "wq0": wq0,
                        "wd": wdc})
    return in_maps


def _emulate(in_maps):
    """Numpy emulation of the device math (bf16 operands, fp32 accum),
    reconstructing operands from the tiled layouts to validate them."""
    f32 = np.float32
    acc = np.zeros((NTOK, H), f32)
    for ci, m in enumerate(in_maps):
        g, d = divmod(ci, DPG)
        xt = m["x"].reshape(NB, NXT, 128, 2, TB)
        xs = xt.transpose(0, 4, 1, 3, 2).reshape(TOKC, H).astype(f32)
        wgc = m["wg"].reshape(C, 128, K, 128)
        wg2 = wgc.transpose(2, 1, 0, 3).reshape(H, IS).astype(f32)
        wuc = m["wu"].reshape(C, 128, K, 128)
        wu2 = wuc.transpose(2, 1, 0, 3).reshape(H, IS).astype(f32)
        wdc = m["wd"].reshape(NHB, 128, C, 128)
        wd2 = wdc.transpose(2, 1, 0, 3).reshape(IS, H).astype(f32)
        y1 = xs @ wg2
        y2 = xs @ wu2
        r = np.maximum(y1, 0).astype(BF16).astype(f32)
        x3 = (r * y2).astype(BF16).astype(f32)
        acc[d * TOKC:(d + 1) * TOKC] += x3 @ wd2
    return acc.reshape(B, S, H)


def kernel(**inputs):
    global LAST_EXEC_TIME_NS, LAST_RESULTS
    in_maps = _prep_inputs(**inputs)

    if os.environ.get("KERNEL_EMULATE"):
        return _emulate(in_maps)

    from concourse.bass_utils import run_bass_kernel_spmd

    nc = _build_nc()
    res = run_bass_kernel_spmd(nc, in_maps, list(range(NCORES)), trace=TRACE)
    LAST_EXEC_TIME_NS = res.exec_time_ns
    LAST_RESULTS = res

    acc = np.zeros((NTOK, H), np.float32)
    for ci, r in enumerate(res.results):
        g, d = divmod(ci, DPG)
        acc[d * TOKC:(d + 1) * TOKC] += r["out"].T
    return acc.reshape(B, S, H)
